# revision 96
# baseline (speedup 1.0000x reference)
"""GAT (3-layer, PyG-style) Trainium2 Bass kernel, 8-core SPMD.

Sharding: nodes are LPT-balanced into 80 tiles of 128 (125 real + 3 pad),
10 tiles per core.  Edges are routed to the (core, tile) owning their
destination, padded to c_max chunks of 128 edges per tile.

Layer 0: every core computes the FULL gather table (xw bf16 | a_src fp32)
from a replicated bf16 copy of x -- no AllGather.  Layers 1/2: the dense
phase for layer L+1 is interleaved tile-by-tile into the edge phase of
layer L, so the AllGather starts the moment the edge phase drains.

Edge phase per dst tile: dma_gather source rows, p = exp(lrelu(a_src +
a_dst)) (a_dst via a PE matmul against a host-streamed transposed
indicator; lrelu/exp on the scalar engine), p broadcast to message width
on the scalar engine, messages scaled in place with packed-bf16 vector
multiplies (2x mode), then indicator matmuls aggregate messages +
denominators into PSUM.  Softmax max-subtraction is skipped (logits are
O(1)).  The elu "-1" is dropped on-device (elu' = elu + 1) and folded
into host-side corrections of b1/b2/a_src (valid since sum(alpha) = 1).
"""
import os
import numpy as np
import ml_dtypes

import concourse.bacc as bacc
import concourse.tile as tile
import concourse.mybir as mybir
from concourse import library_config
from concourse.bass_utils import run_bass_kernel_spmd

NCORES = 8
N = 10000
NLOC = 1280               # padded nodes per core
NPAD = NLOC * NCORES      # 10240
NTILE = NLOC // 128       # 10
NGTILE = NPAD // 128      # 80
N_FEAT = 256
N_HID = 128
N_HEAD = 4
N_HEAD_LAST = 6
N_CLASS = 40
D01 = N_HID * N_HEAD      # 512
D2 = N_HEAD_LAST * N_CLASS  # 240
NEG = 0.2

ROW01 = 640               # uint16 slots per table row, layers 0/1 (1280 B)
ROW2 = 256                # layer 2 (512 B)

# AllGather stripes: each fires the moment its dense tiles are done,
# overlapping the preceding edge phase; the final stripe is a single tile
# so the exposed tail after the edge phase drains is minimal
STRIPE_T0 = (0, 4, 8)          # first tile of each stripe
STRIPE_NT = (4, 4, 2)          # tiles per stripe
STRIPE_BASE = (0, 4096, 8192)


def _stripe_of(t):
    for s in range(len(STRIPE_T0) - 1, -1, -1):
        if t >= STRIPE_T0[s]:
            return s


def _srow(k, t, i):
    """table row of (core k, tile t, slot i) in the striped layout"""
    s = _stripe_of(t)
    return (STRIPE_BASE[s] + k * (STRIPE_NT[s] * 128)
            + (t - STRIPE_T0[s]) * 128 + i)

F32 = mybir.dt.float32
BF16 = mybir.dt.bfloat16
U16 = mybir.dt.uint16
I16 = mybir.dt.int16
AF = mybir.ActivationFunctionType
OP = mybir.AluOpType

LAST_RESULTS = None       # test harness can read exec_time_ns etc.
_PROGRAM_CACHE = {}
_NQ4 = os.environ.get("GAT_NQ1", "") == ""


def _balance_perm(dst):
    """LPT-pack nodes into 80 tiles (125 real + 3 pad each) so per-tile
    edge counts are balanced -> c_max = ceil(max/128) drops 18 -> 17."""
    import heapq
    nb = NCORES * NTILE
    deg = np.bincount(dst, minlength=N).astype(np.int64)
    order = np.argsort(-deg, kind="stable")
    heap = [(0, b) for b in range(nb)]
    heapq.heapify(heap)
    fill = np.zeros(nb, dtype=np.int64)
    P = np.zeros(N, dtype=np.int64)
    for n in order:
        while True:
            load, b = heapq.heappop(heap)
            if fill[b] < 125:
                break
        P[n] = b * 128 + fill[b]
        fill[b] += 1
        heapq.heappush(heap, (load + deg[n], b))
    return P


def _build_program(c_max):
    S = c_max * 128  # edge slots per tile
    nc = bacc.Bacc("TRN2", num_devices=NCORES, debug=False, num_swdge_queues=4)

    # ---------------- kernel I/O ----------------
    xT_in = nc.dram_tensor("xT", [128, 2, NLOC], F32, kind="ExternalInput")
    xTf_in = nc.dram_tensor("xTf", [128, 2, NPAD], BF16, kind="ExternalInput")
    w0f_in = nc.dram_tensor("w0f", [128, 2, D01 + 4], BF16, kind="ExternalInput")
    w0d_in = nc.dram_tensor("w0d", [128, 2, 4], F32, kind="ExternalInput")
    wskip0_in = nc.dram_tensor("wskip0", [128, 2, D01], F32, kind="ExternalInput")
    w1_in = nc.dram_tensor("w1aug", [128, 4, D01 + 8], BF16, kind="ExternalInput")
    w2_in = nc.dram_tensor("w2all", [128, 4, 292], BF16, kind="ExternalInput")
    b0_in = nc.dram_tensor("b0row", [1, D01], F32, kind="ExternalInput")
    b1_in = nc.dram_tensor("b1row", [1, D01], F32, kind="ExternalInput")
    b2_in = nc.dram_tensor("b2row", [1, N_CLASS], F32, kind="ExternalInput")
    c1_in = nc.dram_tensor("c1row", [1, N_HEAD], F32, kind="ExternalInput")
    c2_in = nc.dram_tensor("c2row", [1, N_HEAD_LAST], F32, kind="ExternalInput")
    ones_in = nc.dram_tensor("ones_row", [1, 128], F32, kind="ExternalInput")
    iota_in = nc.dram_tensor("iota_row", [128, 128], F32, kind="ExternalInput")
    identb_in = nc.dram_tensor("identb", [128, 128], BF16, kind="ExternalInput")
    idx_in = nc.dram_tensor("idx_all", [NTILE * 128, S // 16], I16, kind="ExternalInput")
    indt_in = nc.dram_tensor("indt_all", [NTILE * 128, S], BF16, kind="ExternalInput")
    dstloc_in = nc.dram_tensor("dstloc_all", [NTILE * 128, c_max], F32, kind="ExternalInput")
    out_dram = nc.dram_tensor("out_loc", [NLOC, N_CLASS], F32, kind="ExternalOutput")

    tab1_full = nc.dram_tensor("tab1_full", [NPAD, ROW01], U16, addr_space="Shared")
    tab2_full = nc.dram_tensor("tab2_full", [NPAD, ROW2], U16, addr_space="Shared")
    RG = [list(range(NCORES))]

    with tile.TileContext(nc, num_cores=NCORES) as tc:
        with (
            tc.tile_pool(name="persist", bufs=1) as pp,
            tc.tile_pool(name="dram", bufs=1, space="DRAM") as dram,
        ):
            nc.gpsimd.load_library(library_config.mlp)

            # ---- resident constants / weights ----
            def load_const(name, ap, shape, dtype=F32):
                t = pp.tile(shape, dtype, tag=name)
                nc.sync.dma_start(t[:], ap)
                return t

            w0f_sb = load_const("w0f", w0f_in[:], [128, 2, D01 + 4], BF16)

            def tset(nm, shape, dtype):
                return [pp.tile(shape, dtype, tag=f"{nm}_{t}", name=f"{nm}_{t}")
                        for t in range(NTILE)]
            adst0_sb = tset("adst0", [128, 8], BF16)
            adst1_sb = tset("adst1", [128, 8], BF16)
            adst2_sb = tset("adst2", [128, 12], BF16)
            skip0_own = tset("skip0", [128, D01], BF16)
            x1_own = tset("x1own", [128, D01], BF16)
            skip2_own = tset("skip2", [128, N_CLASS], F32)
            x1T_sb = tset("x1T", [128, 4, 128], BF16)
            x2T_sb = x1T_sb          # reuse (x1T[t] dead after dense1 tile t)
            x2_own = skip0_own       # reuse (skip0 dead after layer-0 edge phase)

            tab0_full = dram.tile([NPAD, ROW01], U16, tag="tab0f")
            tab1_bounce = dram.tile([NLOC, ROW01], U16, tag="tb1")
            tab2_bounce = dram.tile([NLOC, ROW2], U16, tag="tb2")

            # =========================================================
            # layer-0 dense: own pass (skip0 + a_dst0, fp32) then full
            # replicated table pass (bf16) -> local tab0_full
            # =========================================================
            with tc.tile_pool(name="xtp", bufs=1) as xtp:
                xT0_sb = xtp.tile([128, 2, NLOC], F32, tag="xT0")
                xTf_sb = xtp.tile([128, 2, NPAD], BF16, tag="xTf")
                # xTf chunks go out FIRST (the full-table pass needs only
                # w0f + xTf); every other constant is needed much later and
                # would otherwise starve the early table writes of DMA.
                # Progressive chunk sizes: a small first chunk lets the PE
                # start ~6us sooner at kernel start.
                c0 = 0
                for w in (640, 640, 1280, 2560, 5120):
                    nc.sync.dma_start(xTf_sb[:, :, c0:c0 + w],
                                      xTf_in[:, :, c0:c0 + w])
                    c0 += w
                w0d_sb = load_const("w0d", w0d_in[:], [128, 2, 4])
                wskip0_sb = load_const("wskip0", wskip0_in[:], [128, 2, D01])
                w1_sb = load_const("w1", w1_in[:], [128, 4, D01 + 8], BF16)
                w2_sb = load_const("w2", w2_in[:], [128, 4, 292], BF16)
                b0row = load_const("b0row", b0_in[:], [1, D01])
                b1row = load_const("b1row", b1_in[:], [1, D01])
                b2row = load_const("b2row", b2_in[:], [1, N_CLASS])
                c1row = load_const("c1row", c1_in[:], [1, N_HEAD])
                c2row = load_const("c2row", c2_in[:], [1, N_HEAD_LAST])
                ones_sb = load_const("ones", ones_in[:], [1, 128])
                iota_sb = load_const("iota", iota_in[:], [128, 1, 128])
                identb_sb = load_const("identb", identb_in[:], [128, 128], BF16)

                with (
                    tc.tile_pool(name="d0d", bufs=4, space="PSUM") as p0d,
                    tc.tile_pool(name="d0a2", bufs=3, space="PSUM") as p0a2,
                    tc.tile_pool(name="d0sl", bufs=6) as sl0,
                ):
                    for g in range(NGTILE):
                        ps_d = p0d.tile([128, D01], F32, tag="ps_d")
                        ps_a = p0a2.tile([128, 4], F32, tag="ps_a2")
                        for k in range(2):
                            lhsT = xTf_sb[:, k, g * 128:(g + 1) * 128]
                            nc.tensor.matmul(out=ps_d[:], lhsT=lhsT, rhs=w0f_sb[:, k, 0:D01],
                                             start=(k == 0), stop=(k == 1))
                            nc.tensor.matmul(out=ps_a[:], lhsT=lhsT, rhs=w0f_sb[:, k, D01:D01 + 4],
                                             start=(k == 0), stop=(k == 1))
                        # row cols [520:640] are never read -> left unwritten
                        slab = sl0.tile([128, 520], U16, tag="slab0")
                        nc.scalar.activation(slab[:, 0:D01].bitcast(BF16), ps_d[:], AF.Copy)
                        nc.vector.tensor_copy(out=slab[:, D01:520].bitcast(F32), in_=ps_a[:])
                        kk, tt = g // NTILE, g % NTILE
                        r0 = _srow(kk, tt, 0)
                        nc.sync.dma_start(tab0_full[r0:r0 + 128, 0:520], slab[:])

                # bias broadcast rows -> [128, D] via K=1 matmul (after the
                # full pass so the table matmuls start immediately)
                with tc.tile_pool(name="psb", bufs=1, space="PSUM") as psb:
                    b_bcast = {}
                    for nm, row, d in (("b0", b0row, D01), ("b1", b1row, D01),
                                       ("b2", b2row, N_CLASS),
                                       ("c1", c1row, N_HEAD), ("c2", c2row, N_HEAD_LAST)):
                        ps = psb.tile([128, d], F32, tag="bias_ps")
                        nc.tensor.matmul(out=ps[:], lhsT=ones_sb[:], rhs=row[:], start=True, stop=True)
                        bb = pp.tile([128, d], F32, tag=f"bb_{nm}")
                        nc.vector.tensor_copy(out=bb[:], in_=ps[:])
                        b_bcast[nm] = bb

                # own pass (skip0 + a_dst0) after the full pass: edge0 waits
                # on the full table, not on these
                nc.sync.dma_start(xT0_sb[:], xT_in[:])
                with (
                    tc.tile_pool(name="d0s", bufs=2, space="PSUM") as p0s,
                    tc.tile_pool(name="d0a", bufs=2, space="PSUM") as p0a,
                ):
                    for t in range(NTILE):
                        ps_s = p0s.tile([128, D01], F32, tag="ps_s")
                        ps_a = p0a.tile([128, 4], F32, tag="ps_a")
                        for k in range(2):
                            lhsT = xT0_sb[:, k, t * 128:(t + 1) * 128]
                            nc.tensor.matmul(out=ps_s[:], lhsT=lhsT, rhs=wskip0_sb[:, k, :],
                                             start=(k == 0), stop=(k == 1))
                            nc.tensor.matmul(out=ps_a[:], lhsT=lhsT, rhs=w0d_sb[:, k, :],
                                             start=(k == 0), stop=(k == 1))
                        # skip0 = x @ Wskip + b0 (bias pre-added)
                        nc.vector.tensor_tensor(out=skip0_own[t][:], in0=ps_s[:],
                                                in1=b_bcast["b0"][:], op=OP.add)
                        nc.vector.tensor_copy(out=adst0_sb[t][:, 0:4], in_=ps_a[:])
                        nc.vector.tensor_tensor(out=adst0_sb[t][:, 4:8], in0=ps_a[:],
                                                in1=adst0_sb[t][:, 0:4], op=OP.subtract)

            # =========================================================
            # dense tiles for layers 1/2 (interleaved into edge phases)
            # =========================================================
            def dense1_tile(t, pd, pa, slp):
                ps_d = pd.tile([128, D01], F32, tag="d1d")
                ps_a = pa.tile([128, 8], F32, tag="d1a")
                for k in range(4):
                    lhsT = x1T_sb[t][:, k, :]
                    nc.tensor.matmul(out=ps_d[:], lhsT=lhsT, rhs=w1_sb[:, k, 0:D01],
                                     start=(k == 0), stop=(k == 3))
                    nc.tensor.matmul(out=ps_a[:], lhsT=lhsT, rhs=w1_sb[:, k, D01:D01 + 8],
                                     start=(k == 0), stop=(k == 3))
                slab = slp.tile([128, 520], U16, tag="slab1")
                nc.scalar.activation(slab[:, 0:D01].bitcast(BF16), ps_d[:], AF.Copy)
                # a_src' - (cs1 + cd1): elu'-shift correction folded here
                nc.vector.tensor_tensor(out=slab[:, D01:520].bitcast(F32),
                                        in0=ps_a[:, 0:4], in1=b_bcast["c1"][:],
                                        op=OP.subtract)
                nc.vector.tensor_copy(out=adst1_sb[t][:, 0:4], in_=ps_a[:, 4:8])
                nc.vector.tensor_tensor(out=adst1_sb[t][:, 4:8], in0=ps_a[:, 4:8],
                                        in1=adst1_sb[t][:, 0:4], op=OP.subtract)
                nc.sync.dma_start(tab1_bounce[t * 128:(t + 1) * 128, 0:520], slab[:])

            def dense2_tile(t, pd, slp):
                ps = pd.tile([128, 292], F32, tag="d2d")
                for k in range(4):
                    nc.tensor.matmul(out=ps[:], lhsT=x2T_sb[t][:, k, :], rhs=w2_sb[:, k, :],
                                     start=(k == 0), stop=(k == 3))
                slab = slp.tile([128, 252], U16, tag="slab2")
                nc.scalar.activation(slab[:, 0:D2].bitcast(BF16), ps[:, 0:D2], AF.Copy)
                nc.vector.tensor_tensor(out=slab[:, D2:252].bitcast(F32),
                                        in0=ps[:, 240:246], in1=b_bcast["c2"][:],
                                        op=OP.subtract)
                nc.vector.tensor_copy(out=adst2_sb[t][:, 0:6], in_=ps[:, 246:252])
                nc.vector.tensor_tensor(out=adst2_sb[t][:, 6:12], in0=ps[:, 246:252],
                                        in1=adst2_sb[t][:, 0:6], op=OP.subtract)
                # skip2 = x2' @ Wskip_out + b2_adj (bias pre-added)
                nc.vector.tensor_tensor(out=skip2_own[t][:], in0=ps[:, 252:292],
                                        in1=b_bcast["b2"][:], op=OP.add)
                nc.sync.dma_start(tab2_bounce[t * 128:(t + 1) * 128, 0:252], slab[:])

            # =========================================================
            # edge phase for layer `lay`
            # =========================================================
            def edge_phase(lay, tab_full, rowlen, dcols, nh, ch, adst_sb,
                           epilogue, post_tile=None, ag_hook=None):
                """Software-pipelined: emission order PRE(t+2), ATT(t+1),
                AGG(t) so each engine's in-order stream has tile t+1's
                independent work between tile t's dependent stages."""
                with (
                    tc.tile_pool(name=f"eg{lay}", bufs=3) as gp,
                    tc.tile_pool(name=f"ei{lay}", bufs=3) as ip,
                    tc.tile_pool(name=f"ea{lay}", bufs=3, space="PSUM") as pagg,
                    tc.tile_pool(name=f"et{lay}", bufs=2, space="PSUM") as padst,
                    tc.tile_pool(name=f"ep{lay}", bufs=1, space="PSUM") as ptr,
                    tc.tile_pool(name=f"ee{lay}", bufs=3) as ep,
                    tc.tile_pool(name=f"er{lay}", bufs=2) as prp,
                ):
                    st = {}

                    def pre(t):
                        rows = slice(t * 128, (t + 1) * 128)
                        idxs = ip.tile([128, S // 16], I16, tag="idx")
                        nc.sync.dma_start(idxs[:], idx_in[rows, :])
                        dstloc = ip.tile([128, c_max, 1], F32, tag="dstloc")
                        nc.sync.dma_start(dstloc[:], dstloc_in[rows, :])
                        indt = ip.tile([128, S], BF16, tag="indt")
                        nc.sync.dma_start(indt[:], indt_in[rows, :])
                        # indicator [e, n]: cheap on DVE, saves streaming it
                        ind = ip.tile([128, c_max, 128], BF16, tag="ind")
                        nc.vector.tensor_tensor(
                            out=ind[:],
                            in0=dstloc[:].to_broadcast([128, c_max, 128]),
                            in1=iota_sb[:].to_broadcast([128, c_max, 128]),
                            op=OP.is_equal)
                        gath = gp.tile([128, c_max, rowlen], U16, tag="gath")
                        ngr = (c_max + 7) // 8  # <=1024 idxs per gather (larger hangs HW)
                        bounds = [round(i * c_max / ngr) for i in range(ngr + 1)]
                        for gi in range(ngr):
                            g0, g1 = bounds[gi], bounds[gi + 1]
                            nidx = (g1 - g0) * 128
                            nc.gpsimd.dma_gather(
                                out_ap=gath[:, g0:g1, :], in_ap=tab_full[:],
                                idxs_ap=idxs[:, g0 * 8:g1 * 8],
                                num_idxs=nidx, num_idxs_reg=nidx, elem_size=rowlen,
                                queue_num=(t * 3 + gi) % 4 if _NQ4 else 0)
                        st[t] = [idxs, ind, indt, gath]

                    def att(t):
                        _, ind, indt, gath = st[t]
                        # a_dst per edge: one small matmul per chunk
                        ps_adst = padst.tile([128, c_max + 1, 2 * nh], F32, tag="ps_adst", name="ps_adst")
                        for c in range(c_max):
                            nc.tensor.matmul(out=ps_adst[:, c, :],
                                             lhsT=indt[:, c * 128:(c + 1) * 128],
                                             rhs=adst_sb[t][:],
                                             start=True, stop=True)

                        # s = a_src + a_dst ; p = exp(lrelu(s))
                        asrc = gath[:, :, dcols:dcols + 2 * nh].bitcast(F32)
                        s_all = ep.tile([128, c_max, nh], F32, tag="s")
                        nc.vector.tensor_tensor(out=s_all[:], in0=ps_adst[:, 0:c_max, 0:nh],
                                                in1=asrc, op=OP.add)
                        nc.vector.tensor_tensor(out=s_all[:], in0=s_all[:],
                                                in1=ps_adst[:, 0:c_max, nh:2 * nh],
                                                op=OP.add)
                        lr = ep.tile([128, c_max, nh], F32, tag="lr")
                        nc.vector.scalar_tensor_tensor(out=lr[:], in0=s_all[:], scalar=NEG,
                                                       in1=s_all[:], op0=OP.mult, op1=OP.max)
                        p_bf = ep.tile([128, c_max, nh, 1], BF16, tag="pbf")
                        nc.scalar.activation(p_bf[:, :, :, 0], lr[:], AF.Exp)

                        # broadcast p to message width on ACT, then scale with
                        # packed-bf16 2x multiplies.  Wide heads: per-head ops
                        # so the multiply of head h overlaps the copy of h+1;
                        # narrow heads (layer 2): single fused ops.
                        p_rep = prp.tile([128, c_max, nh, ch], BF16, tag="prep")
                        if ch >= 128:
                            for h in range(nh):
                                nc.scalar.activation(
                                    p_rep[:, :, h, :],
                                    p_bf[:, :, h, 0:1].to_broadcast([128, c_max, ch]),
                                    AF.Copy)
                            for h in range(nh):
                                mh = gath[:, :, h * ch:(h + 1) * ch].bitcast(BF16)
                                nc.vector.tensor_tensor(out=mh, in0=mh,
                                                        in1=p_rep[:, :, h, :],
                                                        op=OP.mult)
                        else:
                            nc.scalar.activation(
                                p_rep[:],
                                p_bf[:].to_broadcast([128, c_max, nh, ch]),
                                AF.Copy)
                            mh = gath[:, :, 0:dcols].bitcast(BF16)
                            nc.vector.tensor_tensor(out=mh, in0=mh, in1=p_rep[:],
                                                    op=OP.mult)
                        st[t] += [ps_adst, p_bf]

                    def agg(t):
                        _, ind, _, gath, ps_adst, p_bf = st.pop(t)
                        ps_agg = pagg.tile([128, dcols], F32, tag="ps_agg")
                        ps_den = ps_adst[:, c_max, 0:nh]
                        for c in range(c_max):
                            lhsT = ind[:, c, :]
                            nc.tensor.matmul(out=ps_agg[:], lhsT=lhsT,
                                             rhs=gath[:, c, 0:dcols].bitcast(BF16),
                                             start=(c == 0), stop=(c == c_max - 1))
                            nc.tensor.matmul(out=ps_den, lhsT=lhsT,
                                             rhs=p_bf[:, c, :, 0],
                                             start=(c == 0), stop=(c == c_max - 1))
                        epilogue(t, ps_agg, ps_den, ep, ptr)
                        if post_tile is not None:
                            post_tile(t)
                        if ag_hook is not None:
                            ag_hook(t)

                    pre(0)
                    pre(1)
                    att(0)
                    for t in range(NTILE):
                        if t + 2 < NTILE:
                            pre(t + 2)
                        if t + 1 < NTILE:
                            att(t + 1)
                        agg(t)

            def make_next_x(t, ps_agg, ps_den, ep, ptr, skip_src, bias_bc, xout_own, xT_next):
                """v = agg/den + skip (+ bias); x = elu'(v) = max(v,0)+exp(min(v,0));
                write x and its transpose."""
                recip = ep.tile([128, N_HEAD, 1], F32, tag="recip")
                nc.vector.reciprocal(out=recip[:, :, 0], in_=ps_den[:])
                # broadcast 1/den to head width on ACT, scale in ONE vector op
                rrep = ep.tile([128, N_HEAD, N_HID], F32, tag="rrep")
                nc.scalar.activation(rrep[:], recip[:].to_broadcast([128, N_HEAD, N_HID]),
                                     AF.Copy)
                v = ep.tile([128, D01], F32, tag="v")
                nc.vector.tensor_tensor(out=v[:], in0=ps_agg[:], in1=rrep[:],
                                        op=OP.mult)
                nc.vector.tensor_tensor(out=v[:], in0=v[:], in1=skip_src[t][:],
                                        op=OP.add)
                if bias_bc is not None:
                    nc.vector.tensor_tensor(out=v[:], in0=v[:], in1=bias_bc[:],
                                            op=OP.add)
                # eneg = exp(min(v,0)) = Exp(-Relu(-v)), both on ACT
                r = ep.tile([128, D01], F32, tag="r")
                nc.scalar.activation(r[:], v[:], AF.Relu, scale=-1.0)
                eneg = ep.tile([128, D01], F32, tag="eneg")
                nc.scalar.activation(eneg[:], r[:], AF.Exp, scale=-1.0)
                x = xout_own[t][:]
                nc.vector.scalar_tensor_tensor(out=x, in0=v[:], scalar=0.0,
                                               in1=eneg[:], op0=OP.max, op1=OP.add)
                # transpose x tile into xT_next
                for j in range(4):
                    ps_t = ptr.tile([128, 128], BF16, tag="ps_t")
                    nc.tensor.transpose(out=ps_t[:], in_=xout_own[t][:, j * 128:(j + 1) * 128],
                                        identity=identb_sb[:])
                    nc.vector.tensor_copy(out=xT_next[t][:, j, :], in_=ps_t[:])

            # =========================================================
            # layer 0 edges (+ dense1 interleaved), then AG1
            # =========================================================
            def stripe_ag(bounce, full, t):
                """AllGather stripe the moment its dense tiles are done ->
                overlaps the edge phase; final stripe is small (short tail)."""
                if t + 1 in STRIPE_T0 or t == NTILE - 1:
                    s = _stripe_of(t)
                    t0, nt = STRIPE_T0[s], STRIPE_NT[s]
                    rb = slice(t0 * 128, (t0 + nt) * 128)
                    rf = slice(STRIPE_BASE[s], STRIPE_BASE[s] + nt * 128 * NCORES)
                    nc.gpsimd.collective_compute(
                        "AllGather", OP.bypass, replica_groups=RG,
                        ins=[bounce[rb, :].opt()], outs=[full[rf, :].opt()])

            with (
                tc.tile_pool(name="pd1", bufs=1, space="PSUM") as pd1,
                tc.tile_pool(name="pa1", bufs=1, space="PSUM") as pa1,
                tc.tile_pool(name="sl1", bufs=2) as sl1,
            ):
                edge_phase(0, tab0_full, ROW01, D01, N_HEAD, N_HID, adst0_sb,
                           lambda t, pa, pd, ep, ptr: make_next_x(
                               t, pa, pd, ep, ptr, skip0_own, None, x1_own, x1T_sb),
                           post_tile=lambda t: dense1_tile(t, pd1, pa1, sl1),
                           ag_hook=lambda t: stripe_ag(tab1_bounce, tab1_full, t))

            # =========================================================
            # layer 1 edges (+ dense2 interleaved), then AG2
            # =========================================================
            with (
                tc.tile_pool(name="pd2", bufs=1, space="PSUM") as pd2,
                tc.tile_pool(name="sl2", bufs=2) as sl2,
            ):
                edge_phase(1, tab1_full, ROW01, D01, N_HEAD, N_HID, adst1_sb,
                           lambda t, pa, pd, ep, ptr: make_next_x(
                               t, pa, pd, ep, ptr, x1_own, b_bcast["b1"], x2_own, x2T_sb),
                           post_tile=lambda t: dense2_tile(t, pd2, sl2),
                           ag_hook=lambda t: stripe_ag(tab2_bounce, tab2_full, t))

            # =========================================================
            # layer 2 edges -> output
            # =========================================================
            def final_epilogue(t, ps_agg, ps_den, ep, ptr):
                recip = ep.tile([128, N_HEAD_LAST, 1], F32, tag="recip2")
                nc.vector.reciprocal(out=recip[:, :, 0], in_=ps_den[:])
                nc.vector.tensor_scalar_mul(out=recip[:, :, 0], in0=recip[:, :, 0],
                                            scalar1=1.0 / N_HEAD_LAST)
                rrep = ep.tile([128, N_HEAD_LAST, N_CLASS], F32, tag="rrep2")
                nc.scalar.activation(
                    rrep[:], recip[:].to_broadcast([128, N_HEAD_LAST, N_CLASS]), AF.Copy)
                hsc = ep.tile([128, N_HEAD_LAST, N_CLASS], F32, tag="hsc")
                nc.vector.tensor_tensor(out=hsc[:], in0=ps_agg[:], in1=rrep[:], op=OP.mult)
                acc = ep.tile([128, N_CLASS], F32, tag="acc")
                nc.vector.tensor_tensor(out=acc[:], in0=hsc[:, 0, :], in1=hsc[:, 1, :],
                                        op=OP.add)
                for h in range(2, N_HEAD_LAST):
                    nc.vector.tensor_tensor(out=acc[:], in0=acc[:], in1=hsc[:, h, :],
                                            op=OP.add)
                nc.vector.tensor_tensor(out=acc[:], in0=acc[:], in1=skip2_own[t][:],
                                        op=OP.add)
                nc.sync.dma_start(out_dram[t * 128:(t + 1) * 128, :], acc[:])

            edge_phase(2, tab2_full, ROW2, D2, N_HEAD_LAST, N_CLASS, adst2_sb,
                       final_epilogue)

    nc.compile()
    return nc


def _prep_inputs(x, edge_index, W0, a_src0, a_dst0, b0, Wskip_in,
                 W1, a_src1, a_dst1, b1,
                 W2, a_src2, a_dst2, b2, Wskip_out):
    """Host-side routing/layout (no network FLOPs besides weight folding)."""
    x = np.asarray(x, dtype=np.float32)
    ei = np.asarray(edge_index)
    loops = np.arange(N, dtype=np.int64)
    src = np.concatenate([ei[0], loops]).astype(np.int64)
    dst = np.concatenate([ei[1], loops]).astype(np.int64)

    # fold attention vectors into the weight matrices:
    # w_asrc[i, h] = sum_c W[i, h*ch + c] * a_src[h, c]
    def fold(W, a_s, a_d, heads, ch):
        Wr = np.asarray(W, np.float32).reshape(-1, heads, ch)
        ws = np.einsum("ihc,hc->ih", Wr, np.asarray(a_s, np.float32))
        wd = np.einsum("ihc,hc->ih", Wr, np.asarray(a_d, np.float32))
        return np.concatenate([np.asarray(W, np.float32), ws, wd], axis=1)

    w0aug = fold(W0, a_src0, a_dst0, N_HEAD, N_HID)        # [256, 520]
    w1aug = fold(W1, a_src1, a_dst1, N_HEAD, N_HID)        # [512, 520]
    w2aug = fold(W2, a_src2, a_dst2, N_HEAD_LAST, N_CLASS)  # [512, 252]

    W1f = np.asarray(W1, np.float32)
    W2f = np.asarray(W2, np.float32)
    Wso = np.asarray(Wskip_out, np.float32)
    # elu' = elu + 1 corrections (sum(alpha) == 1):
    #  b1' = b1 - colsum(W1) - 1 ;  a_src1 slab -= colsum(ws1)+colsum(wd1)
    #  b2' = b2 - mean_h colsum(W2)_h - colsum(Wskip_out)
    b1row = np.asarray(b1, np.float32) - W1f.sum(0) - 1.0
    c1row = (w1aug[:, D01:D01 + 4].sum(0) + w1aug[:, D01 + 4:D01 + 8].sum(0))
    b2row = (np.asarray(b2, np.float32) - W2f.sum(0).reshape(N_HEAD_LAST, N_CLASS).mean(0)
             - Wso.sum(0))
    c2row = (w2aug[:, D2:D2 + 6].sum(0) + w2aug[:, D2 + 6:D2 + 12].sum(0))

    # ---- edge routing (balanced node->tile assignment) ----
    P = _balance_perm(dst)
    # gather tables use a striped layout so the AllGather can be issued in
    # per-stripe pieces (see _srow)
    kP, lP = P // NLOC, P % NLOC
    tP, iP = lP // 128, lP % 128
    SP = np.array([_srow(k, t, i) for k, t, i in zip(kP, tP, iP)], dtype=np.int64)
    gid = P[dst]
    core = gid // NLOC
    loc = gid - core * NLOC
    tile_id = loc // 128
    dst_local = loc - tile_id * 128
    src_pid = SP[src]

    counts = np.zeros((NCORES, NTILE), dtype=np.int64)
    np.add.at(counts, (core, tile_id), 1)
    c_max = int(np.ceil((counts.max() + 3) / 128))
    S = c_max * 128

    # slot assignment per (core, tile)
    order = np.lexsort((tile_id, core))
    src_s, dl_s, core_s, tile_s = src_pid[order], dst_local[order], core[order], tile_id[order]
    idx_all = np.zeros((NCORES, NTILE, 128, S // 16), dtype=np.int16)
    indt_all = np.zeros((NCORES, NTILE, 128, S), dtype=ml_dtypes.bfloat16)
    dstloc_all = np.full((NCORES, NTILE, 128, c_max), -1.0, dtype=np.float32)
    pos = 0
    for k in range(NCORES):
        for t in range(NTILE):
            cnt = counts[k, t]
            sp = src_s[pos:pos + cnt]
            dl = dl_s[pos:pos + cnt]
            assert (core_s[pos:pos + cnt] == k).all() and (tile_s[pos:pos + cnt] == t).all()
            pos += cnt
            slots_src = np.zeros(S, dtype=np.int16)
            slots_src[:cnt] = sp.astype(np.int16)
            slots_dl = np.full(S, -1.0, dtype=np.float32)
            slots_dl[:cnt] = dl.astype(np.float32)
            # every tile has 3 pad nodes (local 125..127); give each one edge
            # (src = its own zero row) so softmax denominators stay finite
            assert cnt + 3 <= S, (k, t, cnt, S)
            for j in range(3):
                slots_dl[cnt + j] = float(125 + j)
                slots_src[cnt + j] = np.int16(_srow(k, t, 125 + j))
            j = np.arange(S)
            idx_wrapped = np.zeros((16, S // 16), dtype=np.int16)
            idx_wrapped[j % 16, j // 16] = slots_src
            idx_all[k, t] = np.tile(idx_wrapped, (8, 1))
            dstloc_all[k, t, j % 128, j // 128] = slots_dl
            # IndT[n, c*128 + e] = (dst_local of slot (c,e)) == n
            sl2 = slots_dl.reshape(c_max, 128)           # [c, e]
            m = (sl2[None, :, :] == np.arange(128, dtype=np.float32)[:, None, None])
            indt_all[k, t] = m.reshape(128, S).astype(ml_dtypes.bfloat16)

    # ---- x transposes ----
    xpad = np.zeros((NPAD, N_FEAT), dtype=np.float32)
    xpad[P] = x
    xTfull = np.ascontiguousarray(
        xpad.T.reshape(2, 128, NPAD).transpose(1, 0, 2).astype(ml_dtypes.bfloat16))
    xT = np.zeros((NCORES, 128, 2, NLOC), dtype=np.float32)
    for k in range(NCORES):
        xl = xpad[k * NLOC:(k + 1) * NLOC]               # [1280, 256]
        xT[k] = xl.T.reshape(2, 128, NLOC).transpose(1, 0, 2)

    def wlayout(W, kchunks, cols, dtype=np.float32):
        # [in, cols] -> [128, kchunks, cols]
        return np.ascontiguousarray(
            np.asarray(W, np.float32).reshape(kchunks, 128, cols).transpose(1, 0, 2)).astype(dtype)

    w2all = np.concatenate([w2aug, Wso], axis=1)         # [512, 292]

    common = {
        "xTf": xTfull,
        "w0f": wlayout(w0aug[:, 0:D01 + 4], 2, D01 + 4, ml_dtypes.bfloat16),
        "w0d": wlayout(w0aug[:, D01 + 4:D01 + 8], 2, 4),
        "wskip0": wlayout(np.asarray(Wskip_in, np.float32), 2, D01),
        "w1aug": wlayout(w1aug, 4, D01 + 8, ml_dtypes.bfloat16),
        "w2all": wlayout(w2all, 4, 292, ml_dtypes.bfloat16),
        "b0row": np.asarray(b0, np.float32).reshape(1, D01),
        "b1row": b1row.reshape(1, D01).astype(np.float32),
        "b2row": b2row.reshape(1, N_CLASS).astype(np.float32),
        "c1row": c1row.reshape(1, N_HEAD).astype(np.float32),
        "c2row": c2row.reshape(1, N_HEAD_LAST).astype(np.float32),
        "ones_row": np.ones((1, 128), dtype=np.float32),
        "iota_row": np.tile(np.arange(128, dtype=np.float32), (128, 1)),
        "identb": np.eye(128, dtype=ml_dtypes.bfloat16),
    }
    in_maps = []
    for k in range(NCORES):
        m = dict(common)
        m["xT"] = xT[k]
        m["idx_all"] = idx_all[k].reshape(NTILE * 128, S // 16)
        m["indt_all"] = indt_all[k].reshape(NTILE * 128, S)
        m["dstloc_all"] = dstloc_all[k].reshape(NTILE * 128, c_max)
        in_maps.append(m)
    return c_max, in_maps, P


def kernel(**inputs):
    global LAST_RESULTS
    c_max, in_maps, P = _prep_inputs(**inputs)
    if c_max not in _PROGRAM_CACHE:
        _PROGRAM_CACHE[c_max] = _build_program(c_max)
    nc = _PROGRAM_CACHE[c_max]
    import os
    trace = bool(int(os.environ.get("GAT_TRACE", "0")))
    br = run_bass_kernel_spmd(nc, in_maps, list(range(NCORES)), trace=trace)
    LAST_RESULTS = br
    out = np.concatenate([np.asarray(r["out_loc"]) for r in br.results], axis=0)
    return np.ascontiguousarray(out[P], dtype=np.float32)
